# revision 39
# baseline (speedup 1.0000x reference)
"""Multi-head attention (B=2, T=2048, D=1024, H=16) on 8 TRN2 NeuronCores.

Sharding: 2D (batch x head-group). Core c handles batch b = c // 4 and head
group hg = c % 4 (4 heads = 256 channels of the projected dim). Each core:
  1. Projects its batch's q/k against its 256-row weight slices -> QT/KT in
     [j, t] layout (bf16, fp32 PSUM accumulation). V is projected DIRECTLY
     in [t, j] layout (x chunk as the stationary operand), landing in vaug
     augmented with a ones column per head: [V_h | 1] -- no PE transposes.
     Biases are all-zero for this problem's inputs, so the bias matmuls are
     compiled out (with_bias variant kept for generality).  Only K and the
     first half of Q are projected up front; V tiles 0..7 follow, V 8..15
     and Q tiles 2,3 are woven into the attention blocks as PE filler.
  2. Per head pair, per 512-wide q tile: S.T = K_h @ Q_h.T (transposed
     scores), U = exp(S.T * scale) (no max subtraction: |S*scale| <= ~16,
     exp fits fp32 easily), then [O.T ; denom] += [V_h | 1].T @ U -- the
     softmax denominator rides the PV matmul for free as output row 64.
     The PV matmuls trail the score/exp stage by one k tile so the PE
     never waits on ScalarE (keeps the HAM clock at 2.4 GHz).
  3. Raw [O.T ; denom] is staged to SBUF; per-block reciprocals use the
     fast custom DVE op (reciprocal_approx_fast, ~5x cheaper than the
     stock reciprocal); normalization + the output projection for q tile
     qt-1 are woven into the middle of qt's blocks as PE filler.
  4. out_partial.T = woT_chunk.T @ O_norm.T  -> [1024, 2048] fp32.
Host sums the 4 head-group partials per batch, transposes, adds bo.

DMA queue assignment for a fast lead-in: weights on the Sync queue, xk
chunks on GpSimd, xq on Scalar, xv on Vector -- all four stream
concurrently so the K projection starts ~2us in and never starves.

PSUM discipline: exactly one accumulation group per PSUM bank (hardware
start=True clears has_written bits bank-wide). Engine ops only start at
partition offsets {0, 32, 64, 96}; partition shifts (head m=1 belongs at
rows 64-127 of the stage-E operand but results sit at rows 0-64) use
small SBUF->SBUF DMAs.

All shapes are hardcoded for this problem. kernel() takes the full inputs
and returns the full [2, 2048, 1024] fp32 output.
"""

import numpy as np
import ml_dtypes

import concourse.bass as bass
import concourse.bacc as bacc
import concourse.mybir as mybir
import concourse.tile as tile
from concourse.bass_utils import run_bass_kernel_spmd

B, T, D, H, Hd = 2, 2048, 1024, 16, 64
HPC = 4          # heads per core
W = HPC * Hd     # 256 projected channels per core
SCALE = Hd ** -0.5
N_CORES = 8

BF16 = mybir.dt.bfloat16
F32 = mybir.dt.float32
bf16 = ml_dtypes.bfloat16


def build_nc(with_bias=False):
    nc = bacc.Bacc("TRN2", target_bir_lowering=False, debug=False)

    xq = nc.dram_tensor("xq", [D, T], BF16, kind="ExternalInput").ap()
    xk = nc.dram_tensor("xk", [D, T], BF16, kind="ExternalInput").ap()
    xv = nc.dram_tensor("xv", [D, T], BF16, kind="ExternalInput").ap()
    # weights host-preswizzled to [128, chunk, cols] DMA-contiguous layout
    wq = nc.dram_tensor("wq", [128, 8 * W], BF16, kind="ExternalInput").ap()
    wk = nc.dram_tensor("wk", [128, 8 * W], BF16, kind="ExternalInput").ap()
    wv = nc.dram_tensor("wv", [128, 8 * W], BF16, kind="ExternalInput").ap()
    wo = nc.dram_tensor("wo", [128, 2 * D], BF16, kind="ExternalInput").ap()
    if with_bias:
        bq = nc.dram_tensor("bq", [1, W], BF16, kind="ExternalInput").ap()
        bk = nc.dram_tensor("bk", [1, W], BF16, kind="ExternalInput").ap()
        bv = nc.dram_tensor("bv", [1, W], BF16, kind="ExternalInput").ap()
    out = nc.dram_tensor("out", [D, T], F32, kind="ExternalOutput").ap()

    Exp = mybir.ActivationFunctionType.Exp
    import os as _os
    FASTRECIP = _os.environ.get("FASTRECIP", "1") == "1"

    with tile.TileContext(nc) as tc:
        with (
            tc.tile_pool(name="persist", bufs=1) as persist,
            tc.tile_pool(name="xpool", bufs=4) as xpool,
            tc.tile_pool(name="upool", bufs=4) as upool,
            tc.tile_pool(name="rpool", bufs=2) as rpool,
            tc.tile_pool(name="opool", bufs=3) as opool,
        ):
            # ---- input DMAs: ONE queue (sync), strictly in consumption
            # order. HBM BW (~358 GB/s/core) is the shared bottleneck;
            # multiple concurrent queues just split the same bandwidth and
            # delay the early tensors. Stream: wk, xk(in K loop), wq,
            # xq[cols 0:1024], wv, xv, xq[cols 1024:2048], wo.
            wk_sb = persist.tile([128, 8, W], BF16, tag="wk")
            nc.gpsimd.dma_start(out=wk_sb,
                                in_=wk.rearrange("p (c j) -> p c j", j=W))
            wq_sb = persist.tile([128, 8, W], BF16, tag="wq")
            wv_sb = persist.tile([128, 8, W], BF16, tag="wv")
            wo_sb = persist.tile([128, 2, D], BF16, tag="wo")
            xq_sb = persist.tile([128, 8, T], BF16, tag="xq")
            xv_sb = persist.tile([128, 8, T], BF16, tag="xv")
            xq_r = xq.rearrange("(c p) t -> p c t", p=128)
            xv_r = xv.rearrange("(c p) t -> p c t", p=128)

            if with_bias:
                bk_sb = persist.tile([1, W], BF16, tag="bk")
                nc.sync.dma_start(out=bk_sb, in_=bk)
                bq_sb = persist.tile([1, W], BF16, tag="bq")
                nc.sync.dma_start(out=bq_sb, in_=bq)
                bv_sb = persist.tile([1, W], BF16, tag="bv")
                nc.sync.dma_start(out=bv_sb, in_=bv)
                ones_row = persist.tile([1, 512], BF16, tag="ones_row")
                nc.vector.memset(ones_row, 1.0)
                bv_ones = persist.tile([128, 1], BF16, tag="bv_ones")
                nc.vector.memset(bv_ones, 1.0)

            # ---- constants ----
            # K=1 broadcast matmul stationary: ones row at partition 64
            bcast1 = persist.tile([65, 64], BF16, tag="bcast1")
            nc.vector.memset(bcast1[64:65, :], 1.0)

            # ---- persistent activations ----
            qt_sb = persist.tile([128, 2, T], BF16, tag="qt")   # QT [j, t]
            kt_sb = persist.tile([128, 2, T], BF16, tag="kt")   # KT [j, t]
            # V augmented with ones column per head: [k, kt, h, 0:64]=V, [..64]=1
            vaug_sb = persist.tile([128, 16, HPC, Hd + 1], BF16, tag="vaug")
            nc.vector.memset(vaug_sb[:, :, :, 64:65], 1.0)
            otn_sb = persist.tile([128, 2, T], BF16, tag="otn")  # normalized O.T
            # raw [O.T ; denom] per block b2 = (pr*4+qt)*2 + m
            oraw_sb = persist.tile([65, 16, 512], F32, tag="oraw")

            def qk_proj_chunk(pool, w_sb, b_sb, dst, jt, tt, x_res=None,
                              x_dram=None, xcs=None, tag="proj", bufs=None):
                """One [128(j) x 512(t)] projection tile: accumulate 8 chunks."""
                ps = pool.tile([128, 512], F32, tag=tag, bufs=bufs, name="ps")
                for c in range(8):
                    if x_res is not None:
                        xc = x_res[:, c, tt * 512:(tt + 1) * 512]
                    else:
                        xc = xcs[c][:, tt * 512:(tt + 1) * 512]
                    nc.tensor.matmul(
                        ps, lhsT=w_sb[:, c, jt * 128:(jt + 1) * 128], rhs=xc,
                        start=(c == 0), stop=(not with_bias and c == 7),
                    )
                if with_bias:
                    nc.tensor.matmul(
                        ps, lhsT=b_sb[:, jt * 128:(jt + 1) * 128],
                        rhs=ones_row, start=False, stop=True)
                nc.vector.tensor_copy(dst[:, jt, tt * 512:(tt + 1) * 512], ps)

            def v_tile(pool, tt, tag="proj", bufs=None):
                # V tile tt directly in [t, j] layout: stationary = xv chunk
                # slice, moving = wv rows.
                vp = pool.tile([128, 512], F32, tag=tag, bufs=bufs, name="vp")
                for c in range(8):
                    nc.tensor.matmul(
                        vp[:, 0:W],
                        lhsT=xv_sb[:, c, tt * 128:(tt + 1) * 128],
                        rhs=wv_sb[:, c, :],
                        start=(c == 0), stop=(not with_bias and c == 7),
                    )
                if with_bias:
                    nc.tensor.matmul(
                        vp[:, 0:W], lhsT=bv_ones, rhs=bv_sb,
                        start=False, stop=True)
                nc.vector.tensor_copy(
                    vaug_sb[:, tt, :, 0:64],
                    vp[:, 0:W].rearrange("t (h d) -> t h d", h=HPC))

            # ============ Phase A: K, Q(tt 0,1), V(0..7) ============
            with tc.tile_pool(name="psA", bufs=8, space="PSUM") as psA:
                # K: stream xk chunk-by-chunk on the gpsimd (SWDGE) queue --
                # the fast DMA path (~300+ GB/s); HWDGE (sync/scalar)
                # sustains only ~190 GB/s here. All x loads ride gpsimd in
                # strict consumption order; the small weight tensors go on
                # sync concurrently.
                xk_r = xk.rearrange("(c p) t -> p c t", p=128)
                kps = [psA.tile([128, 512], F32, tag="proj", name=f"kp{i}")
                       for i in range(8)]
                for h in range(4):
                    xch = xpool.tile([128, 2, T], BF16, tag="x", name="xch",
                                     bufs=2)
                    nc.gpsimd.dma_start(out=xch,
                                        in_=xk_r[:, 2 * h:2 * h + 2, :])
                    for ci in range(2):
                        c = 2 * h + ci
                        for jt in range(2):
                            for tt in range(4):
                                nc.tensor.matmul(
                                    kps[jt * 4 + tt],
                                    lhsT=wk_sb[:, c, jt * 128:(jt + 1) * 128],
                                    rhs=xch[:, ci, tt * 512:(tt + 1) * 512],
                                    start=(c == 0),
                                    stop=(not with_bias and c == 7),
                                )
                # next pieces of the input stream, in consumption order
                nc.gpsimd.dma_start(out=wq_sb,
                                    in_=wq.rearrange("p (c j) -> p c j", j=W))
                nc.gpsimd.dma_start(out=xq_sb[:, :, 0:1024],
                                    in_=xq_r[:, :, 0:1024])
                nc.gpsimd.dma_start(out=wv_sb,
                                    in_=wv.rearrange("p (c j) -> p c j", j=W))
                for jt in range(2):
                    for tt in range(4):
                        p = kps[jt * 4 + tt]
                        if with_bias:
                            nc.tensor.matmul(
                                p, lhsT=bk_sb[:, jt * 128:(jt + 1) * 128],
                                rhs=ones_row, start=False, stop=True)
                        nc.vector.tensor_copy(
                            kt_sb[:, jt, tt * 512:(tt + 1) * 512], p)

                # Q tiles 0,1 (tiles 2,3 are woven into attention later)
                import os as _os
                WEAVE = _os.environ.get("WEAVE", "1") == "1"
                for h in range(2):
                    nc.gpsimd.dma_start(out=xv_sb[:, 4 * h:4 * h + 4, :],
                                        in_=xv_r[:, 4 * h:4 * h + 4, :])
                for tt in range(2 if WEAVE else 4):
                    for jt in range(2):
                        qk_proj_chunk(psA, wq_sb,
                                      bq_sb if with_bias else None,
                                      qt_sb, jt, tt, x_res=xq_sb)
                nc.gpsimd.dma_start(out=xq_sb[:, :, 1024:2048],
                                    in_=xq_r[:, :, 1024:2048])
                nc.gpsimd.dma_start(out=wo_sb,
                                    in_=wo.rearrange("p (c e) -> p c e", e=D))
                # V tiles 0..7 (8..15 woven into attention): chunk-
                # interleaved wave so the PE trails the xv chunk stream
                # instead of waiting for the whole tensor.
                nv = 8 if WEAVE else 16
                vps = [psA.tile([128, 512], F32, tag="proj", name=f"vp{i}")
                       for i in range(8)]
                for c in range(8):
                    for tt in range(8):
                        nc.tensor.matmul(
                            vps[tt][:, 0:W],
                            lhsT=xv_sb[:, c, tt * 128:(tt + 1) * 128],
                            rhs=wv_sb[:, c, :],
                            start=(c == 0), stop=(not with_bias and c == 7),
                        )
                for tt in range(8):
                    if with_bias:
                        nc.tensor.matmul(
                            vps[tt][:, 0:W], lhsT=bv_ones, rhs=bv_sb,
                            start=False, stop=True)
                    nc.vector.tensor_copy(
                        vaug_sb[:, tt, :, 0:64],
                        vps[tt][:, 0:W].rearrange("t (h d) -> t h d", h=HPC))
                for tt in range(8, nv):
                    v_tile(psA, tt)

            # ====== Phase B/D + fused normalization/output projection ======
            recips = {}

            def attn_block(psB, pr, qt, fillers=()):
                fillers = dict(fillers)
                qsl = slice(qt * 512, (qt + 1) * 512)
                o_psA = psB.tile([65, 512], F32, tag="oA", bufs=1,
                                 name="o_psA")
                o_psB = psB.tile([65, 512], F32, tag="oB", bufs=1,
                                 name="o_psB")
                us = []
                for kt in range(17):
                    if kt < 16:
                        s_big = psB.tile([128, 2, 512], F32, tag="s",
                                         bufs=3, name="s_big")
                        for m in range(2):
                            po = 64 * m
                            nc.tensor.matmul(
                                s_big[:, m, :],
                                lhsT=kt_sb[po:po + 64, pr,
                                           kt * 128:(kt + 1) * 128],
                                rhs=qt_sb[po:po + 64, pr, qsl],
                                start=True, stop=True,
                            )
                        u_big = upool.tile([128, 2, 512], BF16, tag="u",
                                           name="u_big")
                        nc.scalar.activation(u_big, s_big, Exp, scale=SCALE)
                        us.append(u_big)
                    if kt >= 1:
                        for m, o_ps in ((0, o_psA), (1, o_psB)):
                            h = 2 * pr + m
                            nc.tensor.matmul(
                                o_ps,
                                lhsT=vaug_sb[:, kt - 1, h, :],
                                rhs=us[kt - 1][:, m, :],
                                start=(kt == 1), stop=(kt == 16),
                            )
                    # weave prior-tile normalization / projection work into
                    # the loop so ScalarE never starves at block boundaries
                    if kt in fillers:
                        fillers.pop(kt)()
                for fn in fillers.values():
                    fn()
                # stage raw results + per-head reciprocal of the denominator
                # row (partition 64) via the fast custom DVE op, bf16 cast
                # for the broadcast matmul. Normal blocks drain each o_ps
                # bank fully first (frees the bank for the next block's PV
                # ASAP); the final block runs the recip chains before the
                # bulk drains because the tail norm pieces gate on them.
                def drain_m(m, o_ps, what):
                    b2 = (pr * 4 + qt) * 2 + m
                    if what == "bulk":
                        nc.vector.tensor_copy(oraw_sb[0:64, b2, :],
                                              o_ps[0:64, :])
                        return
                    if what == "full":
                        nc.vector.tensor_copy(oraw_sb[:, b2, :], o_ps)
                        return
                    if what == "recip":
                        # denom row not yet staged (last block: bulk drains
                        # run after the recip chains)
                        nc.vector.tensor_copy(oraw_sb[64:65, b2, :],
                                              o_ps[64:65, :])
                    rtb = rpool.tile([65, 512], BF16, tag="rtb", bufs=6,
                                     name="rtb")
                    if FASTRECIP:
                        # the custom DVE op mishandles partition offsets:
                        # run it over the whole 65-row block at offset 0
                        # (same per-lane cost; rows 0:64 are unused junk)
                        rt32 = rpool.tile([65, 512], F32, tag="rt32", bufs=2,
                                          name="rt32")
                        nc.vector.reciprocal_approx_fast(
                            rt32, oraw_sb[:, b2, :])
                        with nc.allow_low_precision(
                                reason="1/denom bf16; ample for softmax"):
                            nc.vector.tensor_copy(rtb[64:65, :],
                                                  rt32[64:65, :])
                    else:
                        with nc.allow_low_precision(
                                reason="1/denom bf16; ample for softmax"):
                            nc.vector.reciprocal(rtb[64:65, :],
                                                 oraw_sb[64:65, b2, :])
                    recips[b2] = rtb

                if qt == 3 and pr == 1:
                    for m, o_ps in ((0, o_psA), (1, o_psB)):
                        drain_m(m, o_ps, "recip")
                    for m, o_ps in ((0, o_psA), (1, o_psB)):
                        drain_m(m, o_ps, "bulk")
                else:
                    # one 65-row copy per bank frees it ASAP for the next
                    # block's PV; recip chains run after both drains
                    for m, o_ps in ((0, o_psA), (1, o_psB)):
                        drain_m(m, o_ps, "full")
                    for m, o_ps in ((0, o_psA), (1, o_psB)):
                        drain_m(m, o_ps, "recip_only")

            def norm_pieces(psB, qt):
                # normalize O.T for q tile qt: 4 filler closures
                qsl = slice(qt * 512, (qt + 1) * 512)

                def piece(pr, m):
                    def run():
                        b2 = (pr * 4 + qt) * 2 + m
                        rb_ps = psB.tile([64, 512], F32, tag="s", bufs=3,
                                         name="rb_ps")
                        nc.tensor.matmul(
                            rb_ps, lhsT=bcast1[64:65, :],
                            rhs=recips[b2][64:65, :],
                            start=True, stop=True)
                        rb_sb = rpool.tile([64, 512], F32, tag="rbs",
                                           name="rb_sb")
                        nc.vector.tensor_copy(rb_sb, rb_ps)
                        if m == 0:
                            nc.vector.tensor_mul(
                                otn_sb[0:64, pr, qsl],
                                oraw_sb[0:64, b2, :], rb_sb)
                        else:
                            otnB = rpool.tile([64, 512], BF16, tag="otnB",
                                              name="otnB")
                            nc.vector.tensor_mul(
                                otnB, oraw_sb[0:64, b2, :], rb_sb)
                            nc.sync.dma_start(
                                out=otn_sb[64:128, pr, qsl], in_=otnB)
                    return run
                # later slots: (pr=1) reciprocals are issued at the
                # immediately preceding block boundary and need time
                return [(9, piece(0, 0)), (11, piece(0, 1)),
                        (13, piece(1, 0)), (15, piece(1, 1))]

            def proj_pieces(psB, qt):
                # output projection for q tile qt: 8 filler closures
                qsl = slice(qt * 512, (qt + 1) * 512)

                def piece(et):
                    def run():
                        e_ps = psB.tile([128, 512], F32, tag="s", bufs=3,
                                        name="e_ps")
                        for jc in range(2):
                            nc.tensor.matmul(
                                e_ps,
                                lhsT=wo_sb[:, jc, et * 128:(et + 1) * 128],
                                rhs=otn_sb[:, jc, qsl],
                                start=(jc == 0), stop=(jc == 1),
                            )
                        stg = opool.tile([128, 512], F32, tag="ostg",
                                         name="stg")
                        nc.vector.tensor_copy(stg, e_ps)
                        nc.sync.dma_start(
                            out=out[et * 128:(et + 1) * 128, qsl], in_=stg)
                    return run
                return [(2 * et + 2, piece(et)) for et in range(8)]

            def q_piece(psB, jt, tt):
                # second-half Q projection (q tiles 2,3) as PE filler
                def run():
                    qk_proj_chunk(psB, wq_sb, bq_sb if with_bias else None,
                                  qt_sb, jt, tt, x_res=xq_sb, tag="s", bufs=3)
                return run

            with tc.tile_pool(name="psB", bufs=1, space="PSUM") as psB:
                for qt in range(4):
                    f0 = list(norm_pieces(psB, qt - 1)) if qt >= 1 else []
                    f1 = list(proj_pieces(psB, qt - 1)) if qt >= 1 else []
                    if WEAVE and qt == 0:
                        # V tiles 8..15, every other slot; v_tile(tt) must
                        # land before PV consumes vaug tile tt at step tt+1
                        # of THIS (pr=0) block -- all deadlines met with
                        # margin: tile 8+i completes during step 2i.
                        f0 += [(2 * i, (lambda t: lambda: v_tile(
                            psB, t, tag="s", bufs=3))(8 + i)) for i in range(8)]
                        # Q tile 2 projection rides qt0's pr=1 block (which
                        # has no other fillers); tile 3 rides qt1's pr=0.
                        f1 += [(2, q_piece(psB, 0, 2)), (4, q_piece(psB, 1, 2))]
                    if WEAVE and qt == 1:
                        f0 += [(1, q_piece(psB, 0, 3)), (3, q_piece(psB, 1, 3))]
                    if qt == 3:
                        # qt3's pr=0 normalization weaves into the pr=1
                        # block (its reciprocals are ready mid-block),
                        # shortening the serial tail.
                        n3 = norm_pieces(psB, 3)
                        f1 += [(13, n3[0][1]), (15, n3[1][1])]
                    attn_block(psB, 0, qt, fillers=f0)
                    attn_block(psB, 1, qt, fillers=f1)
                for _, f in n3[2:]:
                    f()
                for _, f in proj_pieces(psB, 3):
                    f()

    nc.finalize()
    return nc


_NC_CACHE = {}


def _get_nc(with_bias=False):
    if with_bias not in _NC_CACHE:
        _NC_CACHE[with_bias] = build_nc(with_bias)
    return _NC_CACHE[with_bias]


def _swz(wT):
    """[C*128, cols] -> DMA-contiguous [128, C*cols] (partition-major)."""
    C = wT.shape[0] // 128
    return np.ascontiguousarray(
        wT.reshape(C, 128, -1).swapaxes(0, 1).reshape(128, -1)).astype(bf16)


def make_in_maps(query, key, value, wq, bq, wk, bk, wv, bv, wo, bo,
                 with_bias=False):
    in_maps = []
    for c in range(N_CORES):
        b, hg = divmod(c, HPC)
        sl = slice(hg * W, (hg + 1) * W)
        m = {
            "xq": np.ascontiguousarray(np.asarray(query[b]).T).astype(bf16),
            "xk": np.ascontiguousarray(np.asarray(key[b]).T).astype(bf16),
            "xv": np.ascontiguousarray(np.asarray(value[b]).T).astype(bf16),
            "wq": _swz(np.asarray(wq)[sl].T),
            "wk": _swz(np.asarray(wk)[sl].T),
            "wv": _swz(np.asarray(wv)[sl].T),
            "wo": _swz(np.asarray(wo)[:, sl].T),
        }
        if with_bias:
            m["bq"] = np.asarray(bq)[sl].reshape(1, W).astype(bf16)
            m["bk"] = np.asarray(bk)[sl].reshape(1, W).astype(bf16)
            m["bv"] = np.asarray(bv)[sl].reshape(1, W).astype(bf16)
        in_maps.append(m)
    return in_maps


def combine_outputs(outs, bo):
    full = np.zeros((B, T, D), np.float32)
    for c in range(N_CORES):
        b = c // HPC
        full[b] += outs[c].T
    full += np.asarray(bo, np.float32)[None, None, :]
    return full


def kernel(query, key, value, wq, bq, wk, bk, wv, bv, wo, bo):
    with_bias = bool(np.any(np.asarray(bq)) or np.any(np.asarray(bk))
                     or np.any(np.asarray(bv)))
    nc = _get_nc(with_bias)
    in_maps = make_in_maps(query, key, value, wq, bq, wk, bk, wv, bv, wo, bo,
                           with_bias)
    res = run_bass_kernel_spmd(nc, in_maps, list(range(N_CORES)))
    outs = [np.asarray(res.results[c]["out"]) for c in range(N_CORES)]
    return combine_outputs(outs, bo)


# revision 44
# speedup vs baseline: 1.0060x; 1.0060x over previous
"""Multi-head attention (B=2, T=2048, D=1024, H=16) on 8 TRN2 NeuronCores.

Sharding: 2D (batch x head-group). Core c handles batch b = c // 4 and head
group hg = c % 4 (4 heads = 256 channels of the projected dim). Each core:
  1. Projects its batch's q/k against its 256-row weight slices -> QT/KT in
     [j, t] layout (bf16, fp32 PSUM accumulation). V is projected DIRECTLY
     in [t, j] layout (x chunk as the stationary operand), landing in vaug
     augmented with a ones column per head: [V_h | 1] -- no PE transposes.
     Biases are all-zero for this problem's inputs, so the bias matmuls are
     compiled out (with_bias variant kept for generality).  Only K and the
     first half of Q are projected up front; V tiles 0..7 follow, V 8..15
     and Q tiles 2,3 are woven into the attention blocks as PE filler.
  2. Per head pair, per 512-wide q tile: S.T = K_h @ Q_h.T (transposed
     scores), U = exp(S.T * scale) (no max subtraction: |S*scale| <= ~16,
     exp fits fp32 easily), then [O.T ; denom] += [V_h | 1].T @ U -- the
     softmax denominator rides the PV matmul for free as output row 64.
     The PV matmuls trail the score/exp stage by one k tile so the PE
     never waits on ScalarE (keeps the HAM clock at 2.4 GHz).
  3. Raw [O.T ; denom] is staged to SBUF; per-block reciprocals use the
     fast custom DVE op (reciprocal_approx_fast, ~5x cheaper than the
     stock reciprocal); normalization + the output projection for q tile
     qt-1 are woven into the middle of qt's blocks as PE filler.
  4. out_partial.T = woT_chunk.T @ O_norm.T  -> [1024, 2048] fp32.
Host sums the 4 head-group partials per batch, transposes, adds bo.

DMA queue assignment for a fast lead-in: weights on the Sync queue, xk
chunks on GpSimd, xq on Scalar, xv on Vector -- all four stream
concurrently so the K projection starts ~2us in and never starves.

PSUM discipline: exactly one accumulation group per PSUM bank (hardware
start=True clears has_written bits bank-wide). Engine ops only start at
partition offsets {0, 32, 64, 96}; partition shifts (head m=1 belongs at
rows 64-127 of the stage-E operand but results sit at rows 0-64) use
small SBUF->SBUF DMAs.

All shapes are hardcoded for this problem. kernel() takes the full inputs
and returns the full [2, 2048, 1024] fp32 output.
"""

import numpy as np
import ml_dtypes

import concourse.bass as bass
import concourse.bacc as bacc
import concourse.mybir as mybir
import concourse.tile as tile
from concourse.bass_utils import run_bass_kernel_spmd

B, T, D, H, Hd = 2, 2048, 1024, 16, 64
HPC = 4          # heads per core
W = HPC * Hd     # 256 projected channels per core
SCALE = Hd ** -0.5
N_CORES = 8

BF16 = mybir.dt.bfloat16
F32 = mybir.dt.float32
bf16 = ml_dtypes.bfloat16


def build_nc(with_bias=False):
    nc = bacc.Bacc("TRN2", target_bir_lowering=False, debug=False)

    xq = nc.dram_tensor("xq", [D, T], BF16, kind="ExternalInput").ap()
    xk = nc.dram_tensor("xk", [D, T], BF16, kind="ExternalInput").ap()
    xv = nc.dram_tensor("xv", [D, T], BF16, kind="ExternalInput").ap()
    # weights host-preswizzled to [128, chunk, cols] DMA-contiguous layout
    wq = nc.dram_tensor("wq", [128, 8 * W], BF16, kind="ExternalInput").ap()
    wk = nc.dram_tensor("wk", [128, 8 * W], BF16, kind="ExternalInput").ap()
    wv = nc.dram_tensor("wv", [128, 8 * W], BF16, kind="ExternalInput").ap()
    wo = nc.dram_tensor("wo", [128, 2 * D], BF16, kind="ExternalInput").ap()
    if with_bias:
        bq = nc.dram_tensor("bq", [1, W], BF16, kind="ExternalInput").ap()
        bk = nc.dram_tensor("bk", [1, W], BF16, kind="ExternalInput").ap()
        bv = nc.dram_tensor("bv", [1, W], BF16, kind="ExternalInput").ap()
    out = nc.dram_tensor("out", [D, T], F32, kind="ExternalOutput").ap()

    Exp = mybir.ActivationFunctionType.Exp
    import os as _os
    FASTRECIP = _os.environ.get("FASTRECIP", "1") == "1"

    with tile.TileContext(nc) as tc:
        with (
            tc.tile_pool(name="persist", bufs=1) as persist,
            tc.tile_pool(name="xpool", bufs=4) as xpool,
            tc.tile_pool(name="upool", bufs=4) as upool,
            tc.tile_pool(name="rpool", bufs=2) as rpool,
            tc.tile_pool(name="opool", bufs=3) as opool,
        ):
            # ---- input DMAs: ONE queue (sync), strictly in consumption
            # order. HBM BW (~358 GB/s/core) is the shared bottleneck;
            # multiple concurrent queues just split the same bandwidth and
            # delay the early tensors. Stream: wk, xk(in K loop), wq,
            # xq[cols 0:1024], wv, xv, xq[cols 1024:2048], wo.
            wk_sb = persist.tile([128, 8, W], BF16, tag="wk")
            nc.sync.dma_start(out=wk_sb,
                              in_=wk.rearrange("p (c j) -> p c j", j=W))
            wq_sb = persist.tile([128, 8, W], BF16, tag="wq")
            wv_sb = persist.tile([128, 8, W], BF16, tag="wv")
            wo_sb = persist.tile([128, 2, D], BF16, tag="wo")
            xq_sb = persist.tile([128, 8, T], BF16, tag="xq")
            xv_sb = persist.tile([128, 8, T], BF16, tag="xv")
            xq_r = xq.rearrange("(c p) t -> p c t", p=128)
            xv_r = xv.rearrange("(c p) t -> p c t", p=128)

            if with_bias:
                bk_sb = persist.tile([1, W], BF16, tag="bk")
                nc.sync.dma_start(out=bk_sb, in_=bk)
                bq_sb = persist.tile([1, W], BF16, tag="bq")
                nc.sync.dma_start(out=bq_sb, in_=bq)
                bv_sb = persist.tile([1, W], BF16, tag="bv")
                nc.sync.dma_start(out=bv_sb, in_=bv)
                ones_row = persist.tile([1, 512], BF16, tag="ones_row")
                nc.vector.memset(ones_row, 1.0)
                bv_ones = persist.tile([128, 1], BF16, tag="bv_ones")
                nc.vector.memset(bv_ones, 1.0)

            # ---- constants ----
            # K=1 broadcast matmul stationary: ones row at partition 64
            bcast1 = persist.tile([65, 64], BF16, tag="bcast1")
            nc.vector.memset(bcast1[64:65, :], 1.0)

            # ---- persistent activations ----
            qt_sb = persist.tile([128, 2, T], BF16, tag="qt")   # QT [j, t]
            kt_sb = persist.tile([128, 2, T], BF16, tag="kt")   # KT [j, t]
            # V augmented with ones column per head: [k, kt, h, 0:64]=V, [..64]=1
            vaug_sb = persist.tile([128, 16, HPC, Hd + 1], BF16, tag="vaug")
            nc.vector.memset(vaug_sb[:, :, :, 64:65], 1.0)
            otn_sb = persist.tile([128, 2, T], BF16, tag="otn")  # normalized O.T
            # raw [O.T ; denom] per block b2 = (pr*4+qt)*2 + m
            oraw_sb = persist.tile([65, 16, 512], F32, tag="oraw")

            def qk_proj_chunk(pool, w_sb, b_sb, dst, jt, tt, x_res=None,
                              x_dram=None, xcs=None, tag="proj", bufs=None):
                """One [128(j) x 512(t)] projection tile: accumulate 8 chunks."""
                ps = pool.tile([128, 512], F32, tag=tag, bufs=bufs, name="ps")
                for c in range(8):
                    if x_res is not None:
                        xc = x_res[:, c, tt * 512:(tt + 1) * 512]
                    else:
                        xc = xcs[c][:, tt * 512:(tt + 1) * 512]
                    nc.tensor.matmul(
                        ps, lhsT=w_sb[:, c, jt * 128:(jt + 1) * 128], rhs=xc,
                        start=(c == 0), stop=(not with_bias and c == 7),
                    )
                if with_bias:
                    nc.tensor.matmul(
                        ps, lhsT=b_sb[:, jt * 128:(jt + 1) * 128],
                        rhs=ones_row, start=False, stop=True)
                nc.vector.tensor_copy(dst[:, jt, tt * 512:(tt + 1) * 512], ps)

            def v_tile(pool, tt, tag="proj", bufs=None):
                # V tile tt directly in [t, j] layout: stationary = xv chunk
                # slice, moving = wv rows.
                vp = pool.tile([128, 512], F32, tag=tag, bufs=bufs, name="vp")
                for c in range(8):
                    nc.tensor.matmul(
                        vp[:, 0:W],
                        lhsT=xv_sb[:, c, tt * 128:(tt + 1) * 128],
                        rhs=wv_sb[:, c, :],
                        start=(c == 0), stop=(not with_bias and c == 7),
                    )
                if with_bias:
                    nc.tensor.matmul(
                        vp[:, 0:W], lhsT=bv_ones, rhs=bv_sb,
                        start=False, stop=True)
                nc.vector.tensor_copy(
                    vaug_sb[:, tt, :, 0:64],
                    vp[:, 0:W].rearrange("t (h d) -> t h d", h=HPC))

            # ============ Phase A: K, Q(tt 0,1), V(0..7) ============
            with tc.tile_pool(name="psA", bufs=8, space="PSUM") as psA:
                # K: stream xk chunk-by-chunk on the gpsimd (SWDGE) queue --
                # the fast DMA path (~300+ GB/s); HWDGE (sync/scalar)
                # sustains only ~190 GB/s here. All x loads ride gpsimd in
                # strict consumption order; the small weight tensors go on
                # sync concurrently.
                kps = [psA.tile([128, 512], F32, tag="proj", name=f"kp{i}")
                       for i in range(8)]
                for c in range(8):
                    xc = xpool.tile([128, T], BF16, tag="x", name="xc")
                    nc.sync.dma_start(out=xc,
                                      in_=xk[c * 128:(c + 1) * 128, :])
                    for jt in range(2):
                        for tt in range(4):
                            nc.tensor.matmul(
                                kps[jt * 4 + tt],
                                lhsT=wk_sb[:, c, jt * 128:(jt + 1) * 128],
                                rhs=xc[:, tt * 512:(tt + 1) * 512],
                                start=(c == 0), stop=(not with_bias and c == 7),
                            )
                # next pieces of the input stream, in consumption order
                nc.sync.dma_start(out=wq_sb,
                                  in_=wq.rearrange("p (c j) -> p c j", j=W))
                nc.sync.dma_start(out=xq_sb[:, :, 0:1024],
                                  in_=xq_r[:, :, 0:1024])
                nc.sync.dma_start(out=wv_sb,
                                  in_=wv.rearrange("p (c j) -> p c j", j=W))
                for jt in range(2):
                    for tt in range(4):
                        p = kps[jt * 4 + tt]
                        if with_bias:
                            nc.tensor.matmul(
                                p, lhsT=bk_sb[:, jt * 128:(jt + 1) * 128],
                                rhs=ones_row, start=False, stop=True)
                        nc.vector.tensor_copy(
                            kt_sb[:, jt, tt * 512:(tt + 1) * 512], p)

                # Q tiles 0,1 (tiles 2,3 are woven into attention later)
                import os as _os
                WEAVE = _os.environ.get("WEAVE", "1") == "1"
                for c in range(8):
                    nc.sync.dma_start(out=xv_sb[:, c, :], in_=xv_r[:, c, :])
                for tt in range(2 if WEAVE else 4):
                    for jt in range(2):
                        qk_proj_chunk(psA, wq_sb,
                                      bq_sb if with_bias else None,
                                      qt_sb, jt, tt, x_res=xq_sb)
                nc.sync.dma_start(out=xq_sb[:, :, 1024:2048],
                                  in_=xq_r[:, :, 1024:2048])
                nc.sync.dma_start(out=wo_sb,
                                  in_=wo.rearrange("p (c e) -> p c e", e=D))
                # V tiles 0..7 (8..15 woven into attention): chunk-
                # interleaved wave so the PE trails the xv chunk stream
                # instead of waiting for the whole tensor.
                nv = 8 if WEAVE else 16
                vps = [psA.tile([128, 512], F32, tag="proj", name=f"vp{i}")
                       for i in range(8)]
                for c in range(8):
                    for tt in range(8):
                        nc.tensor.matmul(
                            vps[tt][:, 0:W],
                            lhsT=xv_sb[:, c, tt * 128:(tt + 1) * 128],
                            rhs=wv_sb[:, c, :],
                            start=(c == 0), stop=(not with_bias and c == 7),
                        )
                for tt in range(8):
                    if with_bias:
                        nc.tensor.matmul(
                            vps[tt][:, 0:W], lhsT=bv_ones, rhs=bv_sb,
                            start=False, stop=True)
                    nc.vector.tensor_copy(
                        vaug_sb[:, tt, :, 0:64],
                        vps[tt][:, 0:W].rearrange("t (h d) -> t h d", h=HPC))
                for tt in range(8, nv):
                    v_tile(psA, tt)

            # ====== Phase B/D + fused normalization/output projection ======
            recips = {}

            def attn_block(psB, pr, qt, fillers=()):
                fillers = dict(fillers)
                qsl = slice(qt * 512, (qt + 1) * 512)
                o_psA = psB.tile([65, 512], F32, tag="oA", bufs=1,
                                 name="o_psA")
                o_psB = psB.tile([65, 512], F32, tag="oB", bufs=1,
                                 name="o_psB")
                us = []
                for kt in range(17):
                    if kt < 16:
                        s_big = psB.tile([128, 2, 512], F32, tag="s",
                                         bufs=3, name="s_big")
                        for m in range(2):
                            po = 64 * m
                            nc.tensor.matmul(
                                s_big[:, m, :],
                                lhsT=kt_sb[po:po + 64, pr,
                                           kt * 128:(kt + 1) * 128],
                                rhs=qt_sb[po:po + 64, pr, qsl],
                                start=True, stop=True,
                            )
                        u_big = upool.tile([128, 2, 512], BF16, tag="u",
                                           name="u_big")
                        nc.scalar.activation(u_big, s_big, Exp, scale=SCALE)
                        us.append(u_big)
                    if kt >= 1:
                        for m, o_ps in ((0, o_psA), (1, o_psB)):
                            h = 2 * pr + m
                            nc.tensor.matmul(
                                o_ps,
                                lhsT=vaug_sb[:, kt - 1, h, :],
                                rhs=us[kt - 1][:, m, :],
                                start=(kt == 1), stop=(kt == 16),
                            )
                    # weave prior-tile normalization / projection work into
                    # the loop so ScalarE never starves at block boundaries
                    if kt in fillers:
                        fillers.pop(kt)()
                for fn in fillers.values():
                    fn()
                # stage raw results + per-head reciprocal of the denominator
                # row (partition 64) via the fast custom DVE op, bf16 cast
                # for the broadcast matmul. Normal blocks drain each o_ps
                # bank fully first (frees the bank for the next block's PV
                # ASAP); the final block runs the recip chains before the
                # bulk drains because the tail norm pieces gate on them.
                def drain_m(m, o_ps, what):
                    b2 = (pr * 4 + qt) * 2 + m
                    if what == "bulk":
                        nc.vector.tensor_copy(oraw_sb[0:64, b2, :],
                                              o_ps[0:64, :])
                        return
                    if what == "full":
                        nc.vector.tensor_copy(oraw_sb[:, b2, :], o_ps)
                        return
                    if what == "recip":
                        # denom row not yet staged (last block: bulk drains
                        # run after the recip chains)
                        nc.vector.tensor_copy(oraw_sb[64:65, b2, :],
                                              o_ps[64:65, :])
                    rtb = rpool.tile([65, 512], BF16, tag="rtb", bufs=6,
                                     name="rtb")
                    if FASTRECIP:
                        # the custom DVE op mishandles partition offsets:
                        # run it over the whole 65-row block at offset 0
                        # (same per-lane cost; rows 0:64 are unused junk)
                        rt32 = rpool.tile([65, 512], F32, tag="rt32", bufs=2,
                                          name="rt32")
                        nc.vector.reciprocal_approx_fast(
                            rt32, oraw_sb[:, b2, :])
                        with nc.allow_low_precision(
                                reason="1/denom bf16; ample for softmax"):
                            nc.vector.tensor_copy(rtb[64:65, :],
                                                  rt32[64:65, :])
                    else:
                        with nc.allow_low_precision(
                                reason="1/denom bf16; ample for softmax"):
                            nc.vector.reciprocal(rtb[64:65, :],
                                                 oraw_sb[64:65, b2, :])
                    recips[b2] = rtb

                if qt == 3 and pr == 1:
                    for m, o_ps in ((0, o_psA), (1, o_psB)):
                        drain_m(m, o_ps, "recip")
                    for m, o_ps in ((0, o_psA), (1, o_psB)):
                        drain_m(m, o_ps, "bulk")
                else:
                    # one 65-row copy per bank frees it ASAP for the next
                    # block's PV; recip chains run after both drains
                    for m, o_ps in ((0, o_psA), (1, o_psB)):
                        drain_m(m, o_ps, "full")
                    for m, o_ps in ((0, o_psA), (1, o_psB)):
                        drain_m(m, o_ps, "recip_only")

            def norm_pieces(psB, qt):
                # normalize O.T for q tile qt: 4 filler closures
                qsl = slice(qt * 512, (qt + 1) * 512)

                def piece(pr, m):
                    def run():
                        b2 = (pr * 4 + qt) * 2 + m
                        rb_ps = psB.tile([64, 512], F32, tag="s", bufs=3,
                                         name="rb_ps")
                        nc.tensor.matmul(
                            rb_ps, lhsT=bcast1[64:65, :],
                            rhs=recips[b2][64:65, :],
                            start=True, stop=True)
                        rb_sb = rpool.tile([64, 512], F32, tag="rbs",
                                           name="rb_sb")
                        nc.vector.tensor_copy(rb_sb, rb_ps)
                        if m == 0:
                            nc.vector.tensor_mul(
                                otn_sb[0:64, pr, qsl],
                                oraw_sb[0:64, b2, :], rb_sb)
                        else:
                            otnB = rpool.tile([64, 512], BF16, tag="otnB",
                                              name="otnB")
                            nc.vector.tensor_mul(
                                otnB, oraw_sb[0:64, b2, :], rb_sb)
                            nc.sync.dma_start(
                                out=otn_sb[64:128, pr, qsl], in_=otnB)
                    return run
                # later slots: (pr=1) reciprocals are issued at the
                # immediately preceding block boundary and need time
                return [(9, piece(0, 0)), (11, piece(0, 1)),
                        (13, piece(1, 0)), (15, piece(1, 1))]

            def proj_pieces(psB, qt):
                # output projection for q tile qt: 8 filler closures
                qsl = slice(qt * 512, (qt + 1) * 512)

                def piece(et):
                    def run():
                        e_ps = psB.tile([128, 512], F32, tag="s", bufs=3,
                                        name="e_ps")
                        for jc in range(2):
                            nc.tensor.matmul(
                                e_ps,
                                lhsT=wo_sb[:, jc, et * 128:(et + 1) * 128],
                                rhs=otn_sb[:, jc, qsl],
                                start=(jc == 0), stop=(jc == 1),
                            )
                        stg = opool.tile([128, 512], F32, tag="ostg",
                                         name="stg")
                        nc.vector.tensor_copy(stg, e_ps)
                        nc.sync.dma_start(
                            out=out[et * 128:(et + 1) * 128, qsl], in_=stg)
                    return run
                return [(2 * et + 2, piece(et)) for et in range(8)]

            def q_piece(psB, jt, tt):
                # second-half Q projection (q tiles 2,3) as PE filler
                def run():
                    qk_proj_chunk(psB, wq_sb, bq_sb if with_bias else None,
                                  qt_sb, jt, tt, x_res=xq_sb, tag="s", bufs=3)
                return run

            with tc.tile_pool(name="psB", bufs=1, space="PSUM") as psB:
                for qt in range(4):
                    f0 = list(norm_pieces(psB, qt - 1)) if qt >= 1 else []
                    f1 = list(proj_pieces(psB, qt - 1)) if qt >= 1 else []
                    if WEAVE and qt == 0:
                        # V tiles 8..15, every other slot; v_tile(tt) must
                        # land before PV consumes vaug tile tt at step tt+1
                        # of THIS (pr=0) block -- all deadlines met with
                        # margin: tile 8+i completes during step 2i.
                        f0 += [(2 * i, (lambda t: lambda: v_tile(
                            psB, t, tag="s", bufs=3))(8 + i)) for i in range(8)]
                        # Q tile 2 projection rides qt0's pr=1 block (which
                        # has no other fillers); tile 3 rides qt1's pr=0.
                        f1 += [(2, q_piece(psB, 0, 2)), (4, q_piece(psB, 1, 2))]
                    if WEAVE and qt == 1:
                        f0 += [(1, q_piece(psB, 0, 3)), (3, q_piece(psB, 1, 3))]
                    if qt == 3:
                        # qt3's pr=0 normalization weaves into the pr=1
                        # block (its reciprocals are ready mid-block),
                        # shortening the serial tail.
                        n3 = norm_pieces(psB, 3)
                        f1 += [(13, n3[0][1]), (15, n3[1][1])]
                    attn_block(psB, 0, qt, fillers=f0)
                    attn_block(psB, 1, qt, fillers=f1)
                for _, f in n3[2:]:
                    f()
                for _, f in proj_pieces(psB, 3):
                    f()

    nc.finalize()
    return nc


_NC_CACHE = {}


def _get_nc(with_bias=False):
    if with_bias not in _NC_CACHE:
        _NC_CACHE[with_bias] = build_nc(with_bias)
    return _NC_CACHE[with_bias]


def _swz(wT):
    """[C*128, cols] -> DMA-contiguous [128, C*cols] (partition-major)."""
    C = wT.shape[0] // 128
    return np.ascontiguousarray(
        wT.reshape(C, 128, -1).swapaxes(0, 1).reshape(128, -1)).astype(bf16)


def make_in_maps(query, key, value, wq, bq, wk, bk, wv, bv, wo, bo,
                 with_bias=False):
    in_maps = []
    for c in range(N_CORES):
        b, hg = divmod(c, HPC)
        sl = slice(hg * W, (hg + 1) * W)
        m = {
            "xq": np.ascontiguousarray(np.asarray(query[b]).T).astype(bf16),
            "xk": np.ascontiguousarray(np.asarray(key[b]).T).astype(bf16),
            "xv": np.ascontiguousarray(np.asarray(value[b]).T).astype(bf16),
            "wq": _swz(np.asarray(wq)[sl].T),
            "wk": _swz(np.asarray(wk)[sl].T),
            "wv": _swz(np.asarray(wv)[sl].T),
            "wo": _swz(np.asarray(wo)[:, sl].T),
        }
        if with_bias:
            m["bq"] = np.asarray(bq)[sl].reshape(1, W).astype(bf16)
            m["bk"] = np.asarray(bk)[sl].reshape(1, W).astype(bf16)
            m["bv"] = np.asarray(bv)[sl].reshape(1, W).astype(bf16)
        in_maps.append(m)
    return in_maps


def combine_outputs(outs, bo):
    full = np.zeros((B, T, D), np.float32)
    for c in range(N_CORES):
        b = c // HPC
        full[b] += outs[c].T
    full += np.asarray(bo, np.float32)[None, None, :]
    return full


def kernel(query, key, value, wq, bq, wk, bk, wv, bv, wo, bo):
    with_bias = bool(np.any(np.asarray(bq)) or np.any(np.asarray(bk))
                     or np.any(np.asarray(bv)))
    nc = _get_nc(with_bias)
    in_maps = make_in_maps(query, key, value, wq, bq, wk, bk, wv, bv, wo, bo,
                           with_bias)
    res = run_bass_kernel_spmd(nc, in_maps, list(range(N_CORES)))
    outs = [np.asarray(res.results[c]["out"]) for c in range(N_CORES)]
    return combine_outputs(outs, bo)
